# revision 31
# baseline (speedup 1.0000x reference)
"""Trainium2 Bass kernel for nn_Attention_21878563405851.

Module: kv = x1 @ W_qk (k,v split); q = x2 @ W_v; 8-head attention
(dim_head=64); out @ W_out + b_out.  B=2, N=2048, DIM=512.

Sharding over 8 NeuronCores: core c -> batch b=c//4, query chunk
qc=c%4 (512 queries), ALL 8 heads.  Fully collective-free: the kv
projection is recomputed on each of the 4 cores of a batch group
(cheaper than this fabric's AllGather), and each core's output slice
y[b, qc*512:(qc+1)*512, :] is disjoint.

Per core:
  1. q proj (x2 slice, d-major), v proj (x1 full, key-major with a
     ones column appended per head so the softmax denominator falls
     out of the attnv matmul), k proj (x1 full, d-major).
  2. per head h: dots^T[kt] = k_h @ q_h^T -> exp (ACT, scale folded)
     -> attnv accumulated into [65, 512] PSUM (row 64 = denominator).
     Software-pipelined: dots(kt+1) is emitted before attnv(kt) so the
     in-order PE queue never waits on the ACT exp.
  3. normalization: reciprocal of row 64 (DVE), partition-broadcast
     (GpSimd) to 64 rows, DVE multiply into attn-out (bf16).
  4. out proj per head-pair into PSUM, accumulated in SBUF f32 (bias
     folded into the first accumulation), y^T DMA'd out per dim-group.
"""

import sys

for _p in ("/opt/trn_rl_repo", "/root/.axon_site/_ro/trn_rl_repo"):
    if _p not in sys.path:
        sys.path.insert(0, _p)

import numpy as np
import ml_dtypes

import concourse.bass as bass
import concourse.mybir as mybir
from concourse import tile
from concourse.bacc import Bacc

B, N, DIM = 2, 2048, 512
HEADS, DH = 8, 64
INNER = HEADS * DH
SCALE = DH ** -0.5
NCORES = 8
NQ = 512           # queries per core
NKT = N // 128     # 16 key tiles
NC = DIM // 128    # 4 contraction chunks

BF16 = mybir.dt.bfloat16
F32 = mybir.dt.float32


def build_program():
    nc = Bacc(None, num_devices=NCORES)

    # ---- external I/O (per core), host-prearranged SBUF images ----
    x1T = nc.dram_tensor("x1T", [128, NC * N], BF16, kind="ExternalInput")
    x2T = nc.dram_tensor("x2T", [128, NC * NQ], BF16, kind="ExternalInput")
    wk = nc.dram_tensor("wk", [128, 4 * NC * 128], BF16, kind="ExternalInput")
    wq = nc.dram_tensor("wq", [128, 4 * NC * 128], BF16, kind="ExternalInput")
    wv = nc.dram_tensor("wv", [128, NC * INNER], BF16, kind="ExternalInput")
    wo = nc.dram_tensor("wo", [128, 4 * 4 * 128], BF16, kind="ExternalInput")
    bo = nc.dram_tensor("bo", [128, 4], F32, kind="ExternalInput")
    yT = nc.dram_tensor("yT", [128, 4 * NQ], F32, kind="ExternalOutput")

    with tile.TileContext(nc) as tc:
        with (
            tc.tile_pool(name="xin", bufs=1) as xin,
            tc.tile_pool(name="wts", bufs=1) as wts,
            tc.tile_pool(name="kq", bufs=1) as kqp,
            tc.tile_pool(name="vex", bufs=1) as vexp,
            tc.tile_pool(name="et", bufs=4) as etp,
            tc.tile_pool(name="os", bufs=1) as osp,
            tc.tile_pool(name="ysb", bufs=1) as ysbp,
            tc.tile_pool(name="nrm", bufs=2) as nrmp,
            tc.tile_pool(name="dram", bufs=1, space="DRAM") as dramp,
            # PSUM (8 banks): tag "big" [128,1024] x2 (4 banks) for k-proj
            # halves and dots pairs; tag "s5" [128,512] x2 (2 banks) for
            # q/v proj and out-proj partials; tag "acc" [128,512] x2
            # (2 banks) for the long-lived attnv accumulators.
            tc.tile_pool(name="ps", bufs=1, space="PSUM") as psp,
        ):
            # ---- load inputs ----
            # queue plan: SP: x2T, x1c0, x1c1; ACT: x1c2; SWDGE: wq, wk,
            # x1c3, wv, wo, bo.  q-proj is gated on x2T (SP, first) and wq
            # (SWDGE, first); x1 spreads over three queues.
            x2T_s = xin.tile([128, NC * NQ], BF16, name="x2T_s")
            nc.sync.dma_start(x2T_s[:], x2T[:])
            wq_s = wts.tile([128, 4 * NC * 128], BF16, name="wq_s")
            nc.gpsimd.dma_start(wq_s[:], wq[:])
            wk_s = wts.tile([128, 4 * NC * 128], BF16, name="wk_s")
            nc.gpsimd.dma_start(wk_s[:], wk[:])
            x1T_s = xin.tile([128, NC * N], BF16, name="x1T_s")
            x1_eng = [nc.sync, nc.sync, nc.scalar, nc.gpsimd]
            for c in range(NC):
                x1_eng[c].dma_start(
                    x1T_s[:, c * N:(c + 1) * N], x1T[:, c * N:(c + 1) * N]
                )
            wv_s = wts.tile([128, NC * INNER], BF16, name="wv_s")
            nc.gpsimd.dma_start(wv_s[:], wv[:])
            wo_s = wts.tile([128, 4 * 4 * 128], BF16, name="wo_s")
            nc.gpsimd.dma_start(wo_s[:], wo[:])
            bo_s = wts.tile([128, 4], F32, name="bo_s")
            nc.gpsimd.dma_start(bo_s[:], bo[:])

            qT_s = kqp.tile([128, 4 * NQ], BF16, name="qT_s")
            kT_s = kqp.tile([128, 4 * N], BF16, name="kT_s")
            # v extended: per key tile, per head: 64 v cols + 1 ones col
            vE_s = vexp.tile([128, NKT, HEADS, 65], BF16, name="vE_s")
            nc.vector.memset(vE_s[:, :, :, 64:65], 1.0)

            o_s = osp.tile([128, 4, NQ], BF16, name="o_s")
            y_sb = ysbp.tile([128, 4, NQ], F32, name="y_sb")

            # ---- q projection: chunk-outer across all 4 head-pair groups
            # (4 resident PSUM tiles: the s5 pair + the still-idle acc pair)
            # so the first matmul only needs x2T chunk 0 + wq chunk 0 ----
            qps = [
                psp.tile([128, NQ], F32, name=f"psq{g}",
                         tag=("s5" if g < 2 else "acc"), bufs=2)
                for g in range(4)
            ]
            for c in range(NC):
                for g in range(4):
                    nc.tensor.matmul(
                        qps[g][:],
                        wq_s[:, (c * 4 + g) * 128:(c * 4 + g + 1) * 128],
                        x2T_s[:, c * NQ:(c + 1) * NQ],
                        start=(c == 0),
                        stop=(c == NC - 1),
                    )
            for g in range(4):
                nc.vector.tensor_copy(qT_s[:, g * NQ:(g + 1) * NQ], qps[g][:])

            # ---- k projection (d-major): [128 (2h,d), 2048] per group ----
            for g in range(4):
                halves = [
                    psp.tile([128, 1024], F32, name=f"psk{g}{i}", tag="big", bufs=2)
                    for i in range(2)
                ]
                for c in range(NC):
                    for half in range(2):
                        for j in range(2):
                            col = half * 1024 + j * 512
                            nc.tensor.matmul(
                                halves[half][:, j * 512:(j + 1) * 512],
                                wk_s[:, (g * NC + c) * 128:(g * NC + c + 1) * 128],
                                x1T_s[:, c * N + col: c * N + col + 512],
                                start=(c == 0),
                                stop=(c == NC - 1),
                            )
                        if c == NC - 1:
                            # drain each half as soon as it completes so the
                            # next group's matmuls aren't blocked on both
                            nc.vector.tensor_copy(
                                kT_s[:, g * N + half * 1024:
                                     g * N + (half + 1) * 1024],
                                halves[half][:],
                            )

            # ---- v projection (key-major): [128 keys, 512 (h,d)] per kt ----
            for kt in range(NKT):
                ps = psp.tile([128, INNER], F32, name="psv", tag="s5", bufs=2)
                for c in range(NC):
                    nc.tensor.matmul(
                        ps[:],
                        x1T_s[:, c * N + kt * 128: c * N + (kt + 1) * 128],
                        wv_s[:, c * INNER:(c + 1) * INNER],
                        start=(c == 0),
                        stop=(c == NC - 1),
                    )
                nc.vector.tensor_copy(
                    vE_s[:, kt, :, 0:64],
                    ps.rearrange("p (h d) -> p h d", h=HEADS),
                )

            # ---- attention; cross-engine emits deferred so the in-order
            # PE queue never waits on DVE/ACT results ----
            deferred = []
            for h in range(HEADS):
                g, hl = h // 2, h % 2
                r0 = hl * 64
                acc = psp.tile([128, NQ], F32, name=f"acc{h}", tag="acc", bufs=2)

                def emit_attnv(kp, e_t, acc=acc, h=h):
                    for j in range(2):
                        kt = 2 * kp + j
                        nc.tensor.matmul(
                            acc[0:65, :],
                            vE_s[:, kt, h, :],
                            e_t[:, j * 512:(j + 1) * 512],
                            start=(kt == 0),
                            stop=(kt == NKT - 1),
                        )

                # 2-deep software pipeline: attnv(kp-2) is emitted after
                # dots(kp), so the in-order PE always has ~1.8us of queued
                # work while exp(kp-1) (1.1us) runs on ACT.
                pend2 = []
                for kp in range(NKT // 2):
                    dt = psp.tile([128, 1024], F32, name="dt", tag="big", bufs=2)
                    for j in range(2):
                        kt = 2 * kp + j
                        nc.tensor.matmul(
                            dt[:, j * 512:(j + 1) * 512],
                            kT_s[r0:r0 + 64, g * N + kt * 128: g * N + (kt + 1) * 128],
                            qT_s[r0:r0 + 64, g * NQ:(g + 1) * NQ],
                        )
                    e_t = etp.tile([128, 1024], BF16, name="e_t", tag="e")
                    nc.scalar.activation(
                        e_t[:], dt[:],
                        mybir.ActivationFunctionType.Exp, scale=SCALE,
                    )
                    if kp == 1 and deferred:
                        deferred.pop(0)()  # mult of the previous head
                    if kp == 3 and deferred:
                        for fn in deferred:
                            fn()
                        deferred = []
                    if len(pend2) == 2:
                        emit_attnv(*pend2.pop(0))
                    pend2.append((kp, e_t))
                for p2 in pend2:
                    emit_attnv(*p2)

                # normalization: reciprocal now (DVE); broadcast (GpSimd) +
                # multiply (DVE) + out-proj (PE) deferred into next head
                # reciprocal_approx_fast (custom DVE ucode) cannot read PSUM
                # on hw — copy the denominator row to SBUF first.
                s_s = nrmp.tile([1, NQ], F32, name="s_s", tag="s")
                nc.vector.tensor_copy(s_s[:], acc[64:65, :])
                r_s = nrmp.tile([1, NQ], F32, name="r_s", tag="r")
                nc.vector.reciprocal_approx_fast(r_s[:], s_s[:])
                r16 = nrmp.tile([1, NQ], BF16, name="r16", tag="r16")
                nc.vector.tensor_copy(r16[:], r_s[:])
                rb_s = nrmp.tile([64, NQ], BF16, name="rb_s", tag="rb")
                nc.gpsimd.partition_broadcast(rb_s[:], r16[:])

                def emit_mult(acc=acc, rb_s=rb_s, g=g, hl=hl):
                    nc.vector.tensor_mul(
                        o_s[hl * 64:(hl + 1) * 64, g, :], acc[0:64, :], rb_s[:]
                    )
                deferred.append(emit_mult)

                if hl == 1:
                    def emit_y(p=g):
                        for dg in range(4):
                            yp = psp.tile(
                                [128, NQ], F32, name=f"yp{p}{dg}", tag="s5", bufs=2
                            )
                            nc.tensor.matmul(
                                yp[:],
                                wo_s[:, (dg * 4 + p) * 128:(dg * 4 + p + 1) * 128],
                                o_s[:, p, :],
                            )
                            if p == 0:
                                nc.vector.tensor_scalar_add(
                                    y_sb[:, dg, :], yp[:], bo_s[:, dg:dg + 1]
                                )
                            else:
                                nc.vector.tensor_tensor(
                                    y_sb[:, dg, :], y_sb[:, dg, :], yp[:],
                                    mybir.AluOpType.add,
                                )
                    deferred.append(emit_y)

            # flush remaining deferred work (last head's norm + out-proj)
            for fn in deferred:
                fn()

            # ---- final output DMA (spread across queues) ----
            for dg, eng in enumerate(
                (nc.sync, nc.scalar, nc.sync, nc.scalar)
            ):
                eng.dma_start(yT[:, dg * NQ:(dg + 1) * NQ], y_sb[:, dg, :])

    nc.finalize()
    return nc


_NC_CACHE = None


def _get_program():
    global _NC_CACHE
    if _NC_CACHE is None:
        _NC_CACHE = build_program()
    return _NC_CACHE


def make_in_maps(x1, x2, W_qk, W_v, W_out, b_out):
    bf = ml_dtypes.bfloat16
    x1 = np.asarray(x1, np.float32)
    x2 = np.asarray(x2, np.float32)
    W_qk = np.asarray(W_qk, np.float32)
    W_v = np.asarray(W_v, np.float32)
    W_out = np.asarray(W_out, np.float32)
    b_out = np.asarray(b_out, np.float32)

    # weight images, shared by all cores
    # wk/wq: [p, (g c) f] = W[c*128+p, g*128+f]
    def stat_img(W):
        return np.ascontiguousarray(
            W.reshape(NC, 128, 4, 128).transpose(1, 2, 0, 3).reshape(128, 2048)
        ).astype(bf)

    wk_img = stat_img(W_qk[:, :INNER])
    # wq is c-major: [p, (c g) f] so each 512-col chunk is one DMA piece
    wq_img = np.ascontiguousarray(
        W_v.reshape(NC, 128, 4, 128).transpose(1, 0, 2, 3).reshape(128, 2048)
    ).astype(bf)
    # wv: [p, c f] = W_qk[c*128+p, 512+f]
    wv_img = np.ascontiguousarray(
        W_qk[:, INNER:].reshape(NC, 128, INNER).transpose(1, 0, 2).reshape(128, NC * INNER)
    ).astype(bf)
    # wo: [p, (dg pp) f] = W_out[pp*128+p, dg*128+f]
    wo_img = np.ascontiguousarray(
        W_out.reshape(4, 128, 4, 128).transpose(1, 2, 0, 3).reshape(128, 2048)
    ).astype(bf)
    bo_img = np.ascontiguousarray(b_out.reshape(4, 128).T)

    x1T_imgs = [
        np.ascontiguousarray(
            x1[b].reshape(N, NC, 128).transpose(2, 1, 0).reshape(128, NC * N)
        ).astype(bf)
        for b in range(B)
    ]

    in_maps = []
    for c in range(NCORES):
        b, qc = c // 4, c % 4
        qs = qc * NQ
        x2T_img = np.ascontiguousarray(
            x2[b, qs:qs + NQ].reshape(NQ, NC, 128).transpose(2, 1, 0).reshape(128, NC * NQ)
        ).astype(bf)
        in_maps.append(
            {
                "x1T": x1T_imgs[b],
                "x2T": x2T_img,
                "wk": wk_img,
                "wq": wq_img,
                "wv": wv_img,
                "wo": wo_img,
                "bo": bo_img,
            }
        )
    return in_maps


def assemble_output(results):
    y = np.empty((B, N, DIM), np.float32)
    for c in range(NCORES):
        b, qc = c // 4, c % 4
        yTc = np.asarray(results[c]["yT"])  # [128, 4*512]
        D = yTc.reshape(128, 4, NQ).transpose(1, 0, 2).reshape(DIM, NQ)
        y[b, qc * NQ:(qc + 1) * NQ, :] = D.T
    return y


def kernel(x1, x2, W_qk, W_v, W_out, b_out):
    from concourse.bass_utils import run_bass_kernel_spmd

    nc = _get_program()
    in_maps = make_in_maps(x1, x2, W_qk, W_v, W_out, b_out)
    res = run_bass_kernel_spmd(nc, in_maps, list(range(NCORES)))
    return assemble_output(res.results)


# revision 32
# speedup vs baseline: 1.0121x; 1.0121x over previous
"""Trainium2 Bass kernel for nn_Attention_21878563405851.

Module: kv = x1 @ W_qk (k,v split); q = x2 @ W_v; 8-head attention
(dim_head=64); out @ W_out + b_out.  B=2, N=2048, DIM=512.

Sharding over 8 NeuronCores: core c -> batch b=c//4, query chunk
qc=c%4 (512 queries), ALL 8 heads.  Fully collective-free: the kv
projection is recomputed on each of the 4 cores of a batch group
(cheaper than this fabric's AllGather), and each core's output slice
y[b, qc*512:(qc+1)*512, :] is disjoint.

Per core:
  1. q proj (x2 slice, chunk-outer across all 4 head-pair groups so it
     starts as soon as the first input pieces land), k proj (x1 full,
     d-major), v proj (x1 full, key-major with a ones column appended
     per head so the softmax denominator falls out of the attnv
     matmul).  No transposes anywhere.
  2. per head h: dots^T[kt] = k_h @ q_h^T -> exp (ACT, [128,1024]
     pair tiles, scale folded) -> attnv accumulated into [65, 512]
     PSUM (row 64 = denominator).  2-deep software pipeline: attnv
     (kp-2) is emitted after dots(kp) so the in-order PE queue always
     holds ~1.8us of ready work while the 1.1us exp runs on ACT.
  3. normalization: denominator row copied to SBUF (the custom-ucode
     reciprocal cannot read PSUM on hw), reciprocal_approx_fast,
     bf16 convert, partition-broadcast on the idle GpSimd engine,
     DVE multiply into attn-out (bf16).  All but the reciprocal are
     deferred into the next head's dots stream so cross-engine waits
     never stall the PE.
  4. out proj per head-pair into PSUM (flushed two heads later),
     accumulated in SBUF f32 with the bias folded into the first
     accumulation; y^T [128, 4x512] f32 DMA'd out per dim-group.

PSUM (8 banks): "big" [128,1024]x2 for k-proj halves + dots pairs;
"s5" [128,512]x2 for q/v proj + out-proj partials; "acc" [128,512]x2
for q-proj (chunk-outer) then the long-lived attnv accumulators.

Measured: ~136us on hw (baseline AllGather-based kernel: ~208-240us).
The PE matmul stream (409 matmuls, 512 moving cols each) runs at
~265-280ns/matmul back-to-back, i.e. at the full-pstate streaming
floor; fp8 DoubleRow was tried and gives NO speedup on this silicon
(same ~450ns/matmul as bf16), and e/v in fp8 fail the 2e-2 gate
(~2.6% each), so everything stays bf16.
"""

import sys

for _p in ("/opt/trn_rl_repo", "/root/.axon_site/_ro/trn_rl_repo"):
    if _p not in sys.path:
        sys.path.insert(0, _p)

import numpy as np
import ml_dtypes

import concourse.mybir as mybir
from concourse import tile
from concourse.bacc import Bacc

B, N, DIM = 2, 2048, 512
HEADS, DH = 8, 64
INNER = HEADS * DH
SCALE = DH ** -0.5
NCORES = 8
NQ = 512           # queries per core
NKT = N // 128     # 16 key tiles
NC = DIM // 128    # 4 contraction chunks

BF16 = mybir.dt.bfloat16
F32 = mybir.dt.float32


def build_program():
    nc = Bacc(None, num_devices=NCORES)

    # ---- external I/O (per core), host-prearranged SBUF images ----
    x1T = nc.dram_tensor("x1T", [128, NC * N], BF16, kind="ExternalInput")
    x2T = nc.dram_tensor("x2T", [128, NC * NQ], BF16, kind="ExternalInput")
    wk = nc.dram_tensor("wk", [128, 4 * NC * 128], BF16, kind="ExternalInput")
    wq = nc.dram_tensor("wq", [128, 4 * NC * 128], BF16, kind="ExternalInput")
    wv = nc.dram_tensor("wv", [128, NC * INNER], BF16, kind="ExternalInput")
    wo = nc.dram_tensor("wo", [128, 4 * 4 * 128], BF16, kind="ExternalInput")
    bo = nc.dram_tensor("bo", [128, 4], F32, kind="ExternalInput")
    yT = nc.dram_tensor("yT", [128, 4 * NQ], F32, kind="ExternalOutput")

    with tile.TileContext(nc) as tc:
        with (
            tc.tile_pool(name="xin", bufs=1) as xin,
            tc.tile_pool(name="wts", bufs=1) as wts,
            tc.tile_pool(name="kq", bufs=1) as kqp,
            tc.tile_pool(name="vex", bufs=1) as vexp,
            tc.tile_pool(name="et", bufs=4) as etp,
            tc.tile_pool(name="os", bufs=1) as osp,
            tc.tile_pool(name="ysb", bufs=1) as ysbp,
            tc.tile_pool(name="nrm", bufs=2) as nrmp,
            # PSUM (8 banks): tag "big" [128,1024] x2 (4 banks) for k-proj
            # halves and dots pairs; tag "s5" [128,512] x2 (2 banks) for
            # q/v proj and out-proj partials; tag "acc" [128,512] x2
            # (2 banks) for the long-lived attnv accumulators.
            tc.tile_pool(name="ps", bufs=1, space="PSUM") as psp,
        ):
            # ---- load inputs ----
            # queue plan: SP: x2T, x1c0, x1c1; ACT: x1c2; SWDGE: wq, wk,
            # x1c3, wv, wo, bo.  q-proj is gated on x2T (SP, first) and wq
            # (SWDGE, first); x1 spreads over three queues.
            x2T_s = xin.tile([128, NC * NQ], BF16, name="x2T_s")
            nc.sync.dma_start(x2T_s[:], x2T[:])
            wq_s = wts.tile([128, 4 * NC * 128], BF16, name="wq_s")
            nc.gpsimd.dma_start(wq_s[:], wq[:])
            wk_s = wts.tile([128, 4 * NC * 128], BF16, name="wk_s")
            nc.gpsimd.dma_start(wk_s[:], wk[:])
            x1T_s = xin.tile([128, NC * N], BF16, name="x1T_s")
            x1_eng = [nc.sync, nc.sync, nc.scalar, nc.gpsimd]
            for c in range(NC):
                x1_eng[c].dma_start(
                    x1T_s[:, c * N:(c + 1) * N], x1T[:, c * N:(c + 1) * N]
                )
            wv_s = wts.tile([128, NC * INNER], BF16, name="wv_s")
            nc.gpsimd.dma_start(wv_s[:], wv[:])
            wo_s = wts.tile([128, 4 * 4 * 128], BF16, name="wo_s")
            nc.gpsimd.dma_start(wo_s[:], wo[:])
            bo_s = wts.tile([128, 4], F32, name="bo_s")
            nc.gpsimd.dma_start(bo_s[:], bo[:])

            qT_s = kqp.tile([128, 4 * NQ], BF16, name="qT_s")
            kT_s = kqp.tile([128, 4 * N], BF16, name="kT_s")
            # v extended: per key tile, per head: 64 v cols + 1 ones col
            vE_s = vexp.tile([128, NKT, HEADS, 65], BF16, name="vE_s")
            nc.vector.memset(vE_s[:, :, :, 64:65], 1.0)

            o_s = osp.tile([128, 4, NQ], BF16, name="o_s")
            y_sb = ysbp.tile([128, 4, NQ], F32, name="y_sb")

            # ---- q projection: chunk-outer across all 4 head-pair groups
            # (4 resident PSUM tiles: the s5 pair + the still-idle acc pair)
            # so the first matmul only needs x2T chunk 0 + wq chunk 0 ----
            qps = [
                psp.tile([128, NQ], F32, name=f"psq{g}",
                         tag=("s5" if g < 2 else "acc"), bufs=2)
                for g in range(4)
            ]
            for c in range(NC):
                for g in range(4):
                    nc.tensor.matmul(
                        qps[g][:],
                        wq_s[:, (c * 4 + g) * 128:(c * 4 + g + 1) * 128],
                        x2T_s[:, c * NQ:(c + 1) * NQ],
                        start=(c == 0),
                        stop=(c == NC - 1),
                    )
            for g in range(4):
                nc.vector.tensor_copy(qT_s[:, g * NQ:(g + 1) * NQ], qps[g][:])

            # ---- k projection (d-major): [128 (2h,d), 2048] per group ----
            for g in range(4):
                halves = [
                    psp.tile([128, 1024], F32, name=f"psk{g}{i}", tag="big", bufs=2)
                    for i in range(2)
                ]
                for c in range(NC):
                    for half in range(2):
                        for j in range(2):
                            col = half * 1024 + j * 512
                            nc.tensor.matmul(
                                halves[half][:, j * 512:(j + 1) * 512],
                                wk_s[:, (g * NC + c) * 128:(g * NC + c + 1) * 128],
                                x1T_s[:, c * N + col: c * N + col + 512],
                                start=(c == 0),
                                stop=(c == NC - 1),
                            )
                        if c == NC - 1:
                            # drain each half as soon as it completes so the
                            # next group's matmuls aren't blocked on both
                            nc.vector.tensor_copy(
                                kT_s[:, g * N + half * 1024:
                                     g * N + (half + 1) * 1024],
                                halves[half][:],
                            )

            # ---- v projection (key-major): [128 keys, 512 (h,d)] per kt ----
            for kt in range(NKT):
                ps = psp.tile([128, INNER], F32, name="psv", tag="s5", bufs=2)
                for c in range(NC):
                    nc.tensor.matmul(
                        ps[:],
                        x1T_s[:, c * N + kt * 128: c * N + (kt + 1) * 128],
                        wv_s[:, c * INNER:(c + 1) * INNER],
                        start=(c == 0),
                        stop=(c == NC - 1),
                    )
                nc.vector.tensor_copy(
                    vE_s[:, kt, :, 0:64],
                    ps.rearrange("p (h d) -> p h d", h=HEADS),
                )

            # ---- attention; cross-engine emits deferred so the in-order
            # PE queue never waits on DVE/ACT results ----
            deferred = []
            for h in range(HEADS):
                g, hl = h // 2, h % 2
                r0 = hl * 64
                acc = psp.tile([128, NQ], F32, name=f"acc{h}", tag="acc", bufs=2)

                def emit_attnv(kp, e_t, acc=acc, h=h):
                    for j in range(2):
                        kt = 2 * kp + j
                        nc.tensor.matmul(
                            acc[0:65, :],
                            vE_s[:, kt, h, :],
                            e_t[:, j * 512:(j + 1) * 512],
                            start=(kt == 0),
                            stop=(kt == NKT - 1),
                        )

                # 2-deep software pipeline: attnv(kp-2) is emitted after
                # dots(kp), so the in-order PE always has ~1.8us of queued
                # work while exp(kp-1) (1.1us) runs on ACT.
                pend2 = []
                for kp in range(NKT // 2):
                    dt = psp.tile([128, 1024], F32, name="dt", tag="big", bufs=2)
                    for j in range(2):
                        kt = 2 * kp + j
                        nc.tensor.matmul(
                            dt[:, j * 512:(j + 1) * 512],
                            kT_s[r0:r0 + 64, g * N + kt * 128: g * N + (kt + 1) * 128],
                            qT_s[r0:r0 + 64, g * NQ:(g + 1) * NQ],
                        )
                    e_t = etp.tile([128, 1024], BF16, name="e_t", tag="e")
                    nc.scalar.activation(
                        e_t[:], dt[:],
                        mybir.ActivationFunctionType.Exp, scale=SCALE,
                    )
                    if kp == 1 and deferred:
                        deferred.pop(0)()  # mult of the previous head
                    if kp == 3 and deferred:
                        for fn in deferred:
                            fn()
                        deferred = []
                    if len(pend2) == 2:
                        emit_attnv(*pend2.pop(0))
                    pend2.append((kp, e_t))
                for p2 in pend2:
                    emit_attnv(*p2)

                # normalization: reciprocal now (DVE); broadcast (GpSimd) +
                # multiply (DVE) + out-proj (PE) deferred into next head
                # reciprocal_approx_fast (custom DVE ucode) cannot read PSUM
                # on hw — copy the denominator row to SBUF first.
                s_s = nrmp.tile([1, NQ], F32, name="s_s", tag="s")
                nc.vector.tensor_copy(s_s[:], acc[64:65, :])
                r_s = nrmp.tile([1, NQ], F32, name="r_s", tag="r")
                nc.vector.reciprocal_approx_fast(r_s[:], s_s[:])
                r16 = nrmp.tile([1, NQ], BF16, name="r16", tag="r16")
                nc.vector.tensor_copy(r16[:], r_s[:])
                rb_s = nrmp.tile([64, NQ], BF16, name="rb_s", tag="rb")
                nc.gpsimd.partition_broadcast(rb_s[:], r16[:])

                def emit_mult(acc=acc, rb_s=rb_s, g=g, hl=hl):
                    nc.vector.tensor_mul(
                        o_s[hl * 64:(hl + 1) * 64, g, :], acc[0:64, :], rb_s[:]
                    )
                deferred.append(emit_mult)

                if hl == 1:
                    def emit_y(p=g):
                        for dg in range(4):
                            yp = psp.tile(
                                [128, NQ], F32, name=f"yp{p}{dg}", tag="s5", bufs=2
                            )
                            nc.tensor.matmul(
                                yp[:],
                                wo_s[:, (dg * 4 + p) * 128:(dg * 4 + p + 1) * 128],
                                o_s[:, p, :],
                            )
                            if p == 0:
                                nc.vector.tensor_scalar_add(
                                    y_sb[:, dg, :], yp[:], bo_s[:, dg:dg + 1]
                                )
                            else:
                                nc.vector.tensor_tensor(
                                    y_sb[:, dg, :], y_sb[:, dg, :], yp[:],
                                    mybir.AluOpType.add,
                                )
                    deferred.append(emit_y)

            # flush remaining deferred work (last head's norm + out-proj)
            for fn in deferred:
                fn()

            # ---- final output DMA (spread across queues) ----
            for dg, eng in enumerate(
                (nc.sync, nc.scalar, nc.sync, nc.scalar)
            ):
                eng.dma_start(yT[:, dg * NQ:(dg + 1) * NQ], y_sb[:, dg, :])

    nc.finalize()
    return nc


_NC_CACHE = None


def _get_program():
    global _NC_CACHE
    if _NC_CACHE is None:
        _NC_CACHE = build_program()
    return _NC_CACHE


def make_in_maps(x1, x2, W_qk, W_v, W_out, b_out):
    bf = ml_dtypes.bfloat16
    x1 = np.asarray(x1, np.float32)
    x2 = np.asarray(x2, np.float32)
    W_qk = np.asarray(W_qk, np.float32)
    W_v = np.asarray(W_v, np.float32)
    W_out = np.asarray(W_out, np.float32)
    b_out = np.asarray(b_out, np.float32)

    # weight images, shared by all cores
    # wk/wq: [p, (g c) f] = W[c*128+p, g*128+f]
    def stat_img(W):
        return np.ascontiguousarray(
            W.reshape(NC, 128, 4, 128).transpose(1, 2, 0, 3).reshape(128, 2048)
        ).astype(bf)

    wk_img = stat_img(W_qk[:, :INNER])
    # wq is c-major: [p, (c g) f] so each 512-col chunk is one DMA piece
    wq_img = np.ascontiguousarray(
        W_v.reshape(NC, 128, 4, 128).transpose(1, 0, 2, 3).reshape(128, 2048)
    ).astype(bf)
    # wv: [p, c f] = W_qk[c*128+p, 512+f]
    wv_img = np.ascontiguousarray(
        W_qk[:, INNER:].reshape(NC, 128, INNER).transpose(1, 0, 2).reshape(128, NC * INNER)
    ).astype(bf)
    # wo: [p, (dg pp) f] = W_out[pp*128+p, dg*128+f]
    wo_img = np.ascontiguousarray(
        W_out.reshape(4, 128, 4, 128).transpose(1, 2, 0, 3).reshape(128, 2048)
    ).astype(bf)
    bo_img = np.ascontiguousarray(b_out.reshape(4, 128).T)

    x1T_imgs = [
        np.ascontiguousarray(
            x1[b].reshape(N, NC, 128).transpose(2, 1, 0).reshape(128, NC * N)
        ).astype(bf)
        for b in range(B)
    ]

    in_maps = []
    for c in range(NCORES):
        b, qc = c // 4, c % 4
        qs = qc * NQ
        x2T_img = np.ascontiguousarray(
            x2[b, qs:qs + NQ].reshape(NQ, NC, 128).transpose(2, 1, 0).reshape(128, NC * NQ)
        ).astype(bf)
        in_maps.append(
            {
                "x1T": x1T_imgs[b],
                "x2T": x2T_img,
                "wk": wk_img,
                "wq": wq_img,
                "wv": wv_img,
                "wo": wo_img,
                "bo": bo_img,
            }
        )
    return in_maps


def assemble_output(results):
    y = np.empty((B, N, DIM), np.float32)
    for c in range(NCORES):
        b, qc = c // 4, c % 4
        yTc = np.asarray(results[c]["yT"])  # [128, 4*512]
        D = yTc.reshape(128, 4, NQ).transpose(1, 0, 2).reshape(DIM, NQ)
        y[b, qc * NQ:(qc + 1) * NQ, :] = D.T
    return y


def kernel(x1, x2, W_qk, W_v, W_out, b_out):
    from concourse.bass_utils import run_bass_kernel_spmd

    nc = _get_program()
    in_maps = make_in_maps(x1, x2, W_qk, W_v, W_out, b_out)
    res = run_bass_kernel_spmd(nc, in_maps, list(range(NCORES)))
    return assemble_output(res.results)
